# revision 1
# baseline (speedup 1.0000x reference)
"""Trainium2 Bass kernel for nn_BatchGraphEncoder (gnn_message_passing).

Math note: the reference's segment softmax uses B unique segment ids
(groups of size 1), so alpha == exp(x-x)/1 == 1.0 bit-exactly for any
finite scores.  The output is therefore independent of the attention
inputs (w_i, w_j, w_k) and reduces to pure batch sums:

    out[:,   0:128] = sum_b h[b,:]      (broadcast over the N=512 rows)
    out[:, 128:256] = sum_b r[b,:]      (broadcast)
    out[:, 256:384] = sum_b t[b,:,:]    ([512, 128])

This is a memory-bound reduction over B=2048 dominated by reading t
(512 MB).  Strategy: shard B across the 8 cores (data parallel, 64 MB of
t per core), reduce over the local batch on-device, and sum the 8 tiny
[128,512]-shaped partials on the host.

Per-core reduction runs on the TensorEngine: with a stationary matrix
whose column j is all-ones (a sliding window into a [128, 256] tile
that is zero except column 128), lhsT.T @ rhs places the column-sums of
rhs into row j of PSUM and zeros elsewhere, so all 128 column-blocks of
the flattened [256, 65536] shard accumulate into a single [128, 512]
PSUM bank.  DMA: 16 x 4 MB loads with 32 KB contiguous runs.
"""

import numpy as np

B, N, D = 2048, 512, 128
NCORES = 8
B_LOC = B // NCORES          # 256 batch rows per core
FLAT = N * D                 # 65536 flattened (n, d) columns
CHUNKS = B_LOC // 128        # 2 contraction chunks of K=128
GROUPS = 8                   # column groups per chunk
GW = FLAT // GROUPS          # 8192 columns per DMA tile
MMW = 512                    # matmul moving free dim (fp32 max / PSUM bank)
MM = GW // MMW               # 16 matmuls per tile

_BUILT = None
# test.py can inject {"trace": True, ...} here; harness path leaves it empty.
RUN_KWARGS = {}
LAST_RESULTS = None


def _build():
    from concourse import bacc, tile, mybir

    f32 = mybir.dt.float32
    nc = bacc.Bacc(
        "TRN2",
        target_bir_lowering=False,
        debug=False,
        enable_asserts=False,
        num_devices=NCORES,
    )
    t_in = nc.dram_tensor("t_shard", [B_LOC, FLAT], f32, kind="ExternalInput").ap()
    h_in = nc.dram_tensor("h_shard", [B_LOC, D], f32, kind="ExternalInput").ap()
    r_in = nc.dram_tensor("r_shard", [B_LOC, D], f32, kind="ExternalInput").ap()
    out_t = nc.dram_tensor("out_t_part", [128, MMW], f32, kind="ExternalOutput").ap()
    out_hr = nc.dram_tensor("out_hr_part", [2, D], f32, kind="ExternalOutput").ap()

    with tile.TileContext(nc) as tc:
        with (
            tc.tile_pool(name="wconst", bufs=1) as wpool,
            tc.tile_pool(name="loads", bufs=3) as loads,
            tc.tile_pool(name="hr", bufs=4) as hrpool,
            tc.tile_pool(name="res", bufs=1) as res,
            tc.tile_pool(name="acc", bufs=1, space="PSUM") as ppool,
        ):
            # W is zero except column 128 == 1.0; W[:, 128-j : 256-j] is a
            # [128, 128] stationary whose column j is all-ones.
            W = wpool.tile([128, 256], f32)
            nc.vector.memset(W[:], 0.0)
            nc.vector.memset(W[:, 128:129], 1.0)

            psum_t = ppool.tile([128, MMW], f32)
            psum_hr = ppool.tile([128, D], f32)

            # --- h / r batch sums -> rows 0 / 1 of psum_hr ---
            hr_mms = []
            for row, src in ((0, h_in), (1, r_in)):
                for c in range(CHUNKS):
                    ht = hrpool.tile([128, D], f32)
                    nc.sync.dma_start(ht[:], src[128 * c : 128 * (c + 1), :])
                    hr_mms.append((row, ht))
            for i, (row, ht) in enumerate(hr_mms):
                nc.tensor.matmul(
                    psum_hr[:],
                    W[:, 128 - row : 256 - row],
                    ht[:],
                    start=(i == 0),
                    stop=(i == len(hr_mms) - 1),
                )

            # --- t batch sum: colblock j -> row j of psum_t ---
            n_mms = CHUNKS * GROUPS * MM
            i = 0
            for c in range(CHUNKS):
                for g in range(GROUPS):
                    tl = loads.tile([128, GW], f32)
                    nc.sync.dma_start(
                        tl[:],
                        t_in[128 * c : 128 * (c + 1), GW * g : GW * (g + 1)],
                    )
                    for m in range(MM):
                        j = g * MM + m  # psum row == column-block index
                        nc.tensor.matmul(
                            psum_t[:],
                            W[:, 128 - j : 256 - j],
                            tl[:, MMW * m : MMW * (m + 1)],
                            start=(i == 0),
                            stop=(i == n_mms - 1),
                        )
                        i += 1

            res_t = res.tile([128, MMW], f32)
            nc.vector.tensor_copy(res_t[:], psum_t[:])
            nc.sync.dma_start(out_t[:], res_t[:])

            res_hr = res.tile([2, D], f32)
            nc.vector.tensor_copy(res_hr[:], psum_hr[0:2, :])
            nc.sync.dma_start(out_hr[:], res_hr[:])

    nc.compile()
    return nc


def _get_built():
    global _BUILT
    if _BUILT is None:
        _BUILT = _build()
    return _BUILT


def kernel(h, r, t, w_i, w_j, w_k):
    global LAST_RESULTS
    from concourse import bass_utils

    nc = _get_built()
    t2 = np.ascontiguousarray(t, dtype=np.float32).reshape(B, FLAT)
    h = np.ascontiguousarray(h, dtype=np.float32)
    r = np.ascontiguousarray(r, dtype=np.float32)
    in_maps = [
        {
            "t_shard": t2[c * B_LOC : (c + 1) * B_LOC],
            "h_shard": h[c * B_LOC : (c + 1) * B_LOC],
            "r_shard": r[c * B_LOC : (c + 1) * B_LOC],
        }
        for c in range(NCORES)
    ]
    results = bass_utils.run_bass_kernel_spmd(
        nc, in_maps, core_ids=list(range(NCORES)), **RUN_KWARGS
    )
    LAST_RESULTS = results

    sum_t = np.zeros(FLAT, dtype=np.float64)
    sum_h = np.zeros(D, dtype=np.float64)
    sum_r = np.zeros(D, dtype=np.float64)
    for c in range(NCORES):
        sum_t += results.results[c]["out_t_part"].reshape(FLAT)
        sum_h += results.results[c]["out_hr_part"][0]
        sum_r += results.results[c]["out_hr_part"][1]

    out = np.empty((N, 2 * D + D), dtype=np.float32)
    out[:, 0:D] = sum_h.astype(np.float32)[None, :]
    out[:, D : 2 * D] = sum_r.astype(np.float32)[None, :]
    out[:, 2 * D :] = sum_t.astype(np.float32).reshape(N, D)
    return out


# revision 3
# speedup vs baseline: 1.0818x; 1.0818x over previous
"""Trainium2 Bass kernel for nn_BatchGraphEncoder (gnn_message_passing).

Math note: the reference's segment softmax uses B unique segment ids
(groups of size 1), so alpha == exp(x-x)/1 == 1.0 bit-exactly for any
finite scores.  The output is therefore independent of the attention
inputs (w_i, w_j, w_k) and reduces to pure batch sums:

    out[:,   0:128] = sum_b h[b,:]      (broadcast over the N=512 rows)
    out[:, 128:256] = sum_b r[b,:]      (broadcast)
    out[:, 256:384] = sum_b t[b,:,:]    ([512, 128])

This is a memory-bound reduction over B=2048 dominated by reading t
(512 MB).  Strategy: shard B across the 8 cores (data parallel, 64 MB of
t per core), reduce over the local batch on-device, and sum the 8 tiny
[128,512]-shaped partials on the host.

Per-core reduction runs on the TensorEngine: with a stationary matrix
whose column j is all-ones (a sliding window into a [128, 256] tile
that is zero except column 128), lhsT.T @ rhs places the column-sums of
rhs into row j of PSUM and zeros elsewhere, so all 128 column-blocks of
the flattened [256, 65536] shard accumulate into a single [128, 512]
PSUM bank.  DMA: 16 x 4 MB loads with 32 KB contiguous runs.
"""

import numpy as np

B, N, D = 2048, 512, 128
NCORES = 8
B_LOC = B // NCORES          # 256 batch rows per core
FLAT = N * D                 # 65536 flattened (n, d) columns
CHUNKS = B_LOC // 128        # 2 contraction chunks of K=128
GROUPS = 8                   # column groups per chunk
GW = FLAT // GROUPS          # 8192 columns per DMA tile
MMW = 512                    # matmul moving free dim (fp32 max / PSUM bank)
MM = GW // MMW               # 16 matmuls per tile

_BUILT = None
# test.py can inject {"trace": True, ...} here; harness path leaves it empty.
RUN_KWARGS = {}
LAST_RESULTS = None


def _build():
    from concourse import bacc, tile, mybir

    f32 = mybir.dt.float32
    nc = bacc.Bacc(
        "TRN2",
        target_bir_lowering=False,
        debug=False,
        enable_asserts=False,
        num_devices=NCORES,
    )
    t_in = nc.dram_tensor("t_shard", [B_LOC, FLAT], f32, kind="ExternalInput").ap()
    h_in = nc.dram_tensor("h_shard", [B_LOC, D], f32, kind="ExternalInput").ap()
    r_in = nc.dram_tensor("r_shard", [B_LOC, D], f32, kind="ExternalInput").ap()
    out_t = nc.dram_tensor("out_t_part", [128, MMW], f32, kind="ExternalOutput").ap()
    out_hr = nc.dram_tensor("out_hr_part", [2, D], f32, kind="ExternalOutput").ap()

    with tile.TileContext(nc) as tc:
        with (
            tc.tile_pool(name="wconst", bufs=1) as wpool,
            tc.tile_pool(name="loads", bufs=4) as loads,
            tc.tile_pool(name="hr", bufs=4) as hrpool,
            tc.tile_pool(name="res", bufs=1) as res,
            tc.tile_pool(name="acc", bufs=1, space="PSUM") as ppool,
        ):
            # W is zero except column 128 == 1.0; W[:, 128-j : 256-j] is a
            # [128, 128] stationary whose column j is all-ones.
            W = wpool.tile([128, 256], f32)
            nc.vector.memset(W[:], 0.0)
            nc.vector.memset(W[:, 128:129], 1.0)

            psum_t = ppool.tile([128, MMW], f32)
            psum_hr = ppool.tile([128, D], f32)

            # --- h / r batch sums -> rows 0 / 1 of psum_hr ---
            hr_mms = []
            for row, src in ((0, h_in), (1, r_in)):
                for c in range(CHUNKS):
                    ht = hrpool.tile([128, D], f32)
                    nc.sync.dma_start(ht[:], src[128 * c : 128 * (c + 1), :])
                    hr_mms.append((row, ht))
            for i, (row, ht) in enumerate(hr_mms):
                nc.tensor.matmul(
                    psum_hr[:],
                    W[:, 128 - row : 256 - row],
                    ht[:],
                    start=(i == 0),
                    stop=(i == len(hr_mms) - 1),
                )

            # --- t batch sum on the DVE ---
            # Tile layout: partition p holds flat columns [512p, 512p+512);
            # free dim packs NB=16 batch rows of 512 columns each.  The fp32
            # tensor_tensor add consumes one element from each read port per
            # cycle, so in-place halving folds net ~1 raw element/cycle/lane.
            NB = 16
            NT = B_LOC // NB  # 16 tiles of [128, NB*512] = 4 MB
            add = mybir.AluOpType.add
            acc = res.tile([128, 1024], f32)  # width-1024 accumulator
            for k in range(NT):
                tl = loads.tile([128, NB * MMW], f32)
                src = t_in[NB * k : NB * (k + 1), :].rearrange(
                    "b (p c) -> p b c", p=128
                )
                nc.sync.dma_start(tl[:].rearrange("p (b c) -> p b c", b=NB), src)
                nc.vector.tensor_tensor(tl[:, :4096], tl[:, :4096], tl[:, 4096:], add)
                nc.vector.tensor_tensor(tl[:, :2048], tl[:, :2048], tl[:, 2048:4096], add)
                nc.vector.tensor_tensor(tl[:, :1024], tl[:, :1024], tl[:, 1024:2048], add)
                if k == 0:
                    nc.vector.tensor_copy(acc[:], tl[:, :1024])
                else:
                    nc.vector.tensor_tensor(acc[:], acc[:], tl[:, :1024], add)

            res_t = res.tile([128, MMW], f32)
            nc.vector.tensor_tensor(res_t[:], acc[:, :512], acc[:, 512:], add)
            nc.sync.dma_start(out_t[:], res_t[:])

            res_hr = res.tile([2, D], f32)
            nc.vector.tensor_copy(res_hr[:], psum_hr[0:2, :])
            nc.sync.dma_start(out_hr[:], res_hr[:])

    nc.compile()
    return nc


def _get_built():
    global _BUILT
    if _BUILT is None:
        _BUILT = _build()
    return _BUILT


def kernel(h, r, t, w_i, w_j, w_k):
    global LAST_RESULTS
    from concourse import bass_utils

    nc = _get_built()
    t2 = np.ascontiguousarray(t, dtype=np.float32).reshape(B, FLAT)
    h = np.ascontiguousarray(h, dtype=np.float32)
    r = np.ascontiguousarray(r, dtype=np.float32)
    in_maps = [
        {
            "t_shard": t2[c * B_LOC : (c + 1) * B_LOC],
            "h_shard": h[c * B_LOC : (c + 1) * B_LOC],
            "r_shard": r[c * B_LOC : (c + 1) * B_LOC],
        }
        for c in range(NCORES)
    ]
    results = bass_utils.run_bass_kernel_spmd(
        nc, in_maps, core_ids=list(range(NCORES)), **RUN_KWARGS
    )
    LAST_RESULTS = results

    sum_t = np.zeros(FLAT, dtype=np.float64)
    sum_h = np.zeros(D, dtype=np.float64)
    sum_r = np.zeros(D, dtype=np.float64)
    for c in range(NCORES):
        sum_t += results.results[c]["out_t_part"].reshape(FLAT)
        sum_h += results.results[c]["out_hr_part"][0]
        sum_r += results.results[c]["out_hr_part"][1]

    out = np.empty((N, 2 * D + D), dtype=np.float32)
    out[:, 0:D] = sum_h.astype(np.float32)[None, :]
    out[:, D : 2 * D] = sum_r.astype(np.float32)[None, :]
    out[:, 2 * D :] = sum_t.astype(np.float32).reshape(N, D)
    return out
